# revision 1
# baseline (speedup 1.0000x reference)
"""Negative-sampling loss kernel for Trainium2 (8 NeuronCores, SPMD).

Math: for E [16384,128], S = E@E.T,
  loss = -(mean_i [ logsig(S_ii) + sum_j logsig(-(S - diag(S))_ij) ])
       = (T - frob2)/N + ln(2)
where T = sum_ij softplus(S_ij) (full matrix incl. diagonal),
frob2 = sum(E^2), N = 16384.
(Derivation: logsig(-x) = -softplus(x); masked diagonal contributes
logsig(0) = -ln2 per row; softplus(-s) = softplus(s) - s on the diagonal.)

S is symmetric, so T = P1 + 2*P2 over 512x512 super-blocks: P1 = the 32
diagonal blocks (counted once), P2 = 528-32=496 unique off-diagonal blocks.
Circulant assignment: super-row r computes blocks (r, (r+d) mod 32) for
d = 0..16 (d=16 only for r < 16). Core c owns super-rows {c, c+8, c+16, c+24}
-> 66 blocks/core, identical program on every core (SPMD), only data differs.

Device: per block, 4 matmuls [128x128]x[128x512] -> PSUM [128,2048], then
softplus via Exp -> Ln(u*2^-64 + 2^-64) = softplus(x) - 64*ln2 (keeps the
Ln table input inside its [2^-64, 2^64] domain for all x <= 88), accumulated
per-partition by the ACT accum port. Diagonal blocks are clamped at 88 on
the DVE first (s_ii ~ 128 would overflow exp); the host adds back
sum_i relu(s_ii - 88) computed from row norms.
"""

import sys

for _p in ("/opt/trn_rl_repo",):
    if _p not in sys.path:
        sys.path.insert(0, _p)

import numpy as np

N = 16384
D = 128
SB = 512            # super-block edge
NSB = N // SB       # 32 super-rows
NCORES = 8
DS = (17, 17, 16, 16)   # blocks per slot (slots 0,1 hold super-rows 0..15)
GROUPS = [(s, 0) for s in range(4)] + [
    (s, d) for s in range(4) for d in range(1, DS[s])
]                   # 66 groups; diagonal blocks first
NG = len(GROUPS)
GROUP_F = 2048      # free elements per group per partition
LN_SCALE = float(2.0 ** -64)
DIAG_CLAMP = 88.0
LN2 = float(np.log(2.0))

TRACE = False
LAST_EXEC_NS = None
LAST_PROFILE = None

_COMPILED = None


def _build():
    import concourse.bacc as bacc
    import concourse.mybir as mybir
    import concourse.hw_specs as hw_specs
    from concourse import tile
    from contextlib import ExitStack

    AF = mybir.ActivationFunctionType

    # Force Exp and Ln to resolve to the single table set that contains both
    # (natural_log_exp_and_others); otherwise the table-load pass may pick
    # different sets and reload the 828-entry table between instructions.
    _orig = hw_specs.get_activation_tables

    def _patched(arch):
        t = {k: set(v) for k, v in _orig(arch).items()}
        for name in t:
            if name != "natural_log_exp_and_others":
                t[name] -= {AF.Exp, AF.Ln}
        return t

    bacc.get_activation_tables = _patched

    nc = bacc.Bacc("TRN2", target_bir_lowering=False, debug=False,
                   num_devices=NCORES)
    f32 = mybir.dt.float32
    cols_ap = nc.dram_tensor("cols", [D, NG * SB], f32, kind="ExternalInput").ap()
    shard_ap = nc.dram_tensor("shard", [D, 4 * SB], f32, kind="ExternalInput").ap()
    out_ap = nc.dram_tensor("out", [D, NG], f32, kind="ExternalOutput").ap()

    with tile.TileContext(nc) as tc:
        with ExitStack() as ctx:
            const_pool = ctx.enter_context(tc.tile_pool(name="const", bufs=1))
            shard_pool = ctx.enter_context(tc.tile_pool(name="shardp", bufs=1))
            rhs_pool = ctx.enter_context(tc.tile_pool(name="rhs", bufs=4))
            u_pool = ctx.enter_context(tc.tile_pool(name="u", bufs=2))
            o_pool = ctx.enter_context(tc.tile_pool(name="o", bufs=2))
            acc_pool = ctx.enter_context(tc.tile_pool(name="acc", bufs=1))
            psum_pool = ctx.enter_context(
                tc.tile_pool(name="psum", bufs=2, space="PSUM"))

            bias_t = const_pool.tile([D, 1], f32)
            nc.gpsimd.memset(bias_t[:], LN_SCALE)
            shard_t = shard_pool.tile([D, 4 * SB], f32)
            nc.sync.dma_start(shard_t[:], shard_ap)
            acc_t = acc_pool.tile([D, NG], f32)

            for g, (s, d) in enumerate(GROUPS):
                rhs_t = rhs_pool.tile([D, SB], f32)
                nc.sync.dma_start(rhs_t[:], cols_ap[:, g * SB:(g + 1) * SB])
                ps = psum_pool.tile([D, GROUP_F], f32)
                for m in range(4):
                    lo = s * SB + m * D
                    nc.tensor.matmul(
                        ps[:, m * SB:(m + 1) * SB],
                        shard_t[:, lo:lo + D],
                        rhs_t[:],
                        start=True, stop=True,
                    )
                if d == 0:
                    nc.vector.tensor_scalar_min(ps[:], ps[:], DIAG_CLAMP)
                u_t = u_pool.tile([D, GROUP_F], f32)
                nc.scalar.activation(u_t[:], ps[:], AF.Exp)
                o_t = o_pool.tile([D, GROUP_F], f32)
                nc.scalar.activation(
                    o_t[:], u_t[:], AF.Ln,
                    bias=bias_t[:], scale=LN_SCALE,
                    accum_out=acc_t[:, g:g + 1],
                )
            nc.sync.dma_start(out_ap, acc_t[:])

    nc.compile()
    return nc


def kernel(doc_embeddings: np.ndarray) -> np.ndarray:
    global _COMPILED, LAST_EXEC_NS, LAST_PROFILE
    E = np.ascontiguousarray(np.asarray(doc_embeddings, dtype=np.float32))
    assert E.shape == (N, D)
    ET = np.ascontiguousarray(E.T)  # [128, 16384]

    if _COMPILED is None:
        _COMPILED = _build()
    nc = _COMPILED

    in_maps = []
    for c in range(NCORES):
        slots = [c, c + 8, c + 16, c + 24]
        cols_c = np.concatenate(
            [ET[:, ((slots[s] + d) % NSB) * SB:((slots[s] + d) % NSB) * SB + SB]
             for (s, d) in GROUPS], axis=1)
        shard_c = np.concatenate(
            [ET[:, r * SB:r * SB + SB] for r in slots], axis=1)
        in_maps.append({"cols": np.ascontiguousarray(cols_c),
                        "shard": np.ascontiguousarray(shard_c)})

    from concourse import bass_utils
    res = bass_utils.run_bass_kernel_spmd(
        nc, in_maps, core_ids=list(range(NCORES)), trace=TRACE)
    LAST_EXEC_NS = res.exec_time_ns
    LAST_PROFILE = res.profile_json

    # Each accum column summed 128*2048 elements, each offset by -64*ln2.
    off = float(D) * GROUP_F * 64.0 * LN2
    T = 0.0
    for c in range(NCORES):
        colsums = res.results[c]["out"].astype(np.float64).sum(axis=0)
        V = colsums + off
        T += V[:4].sum() + 2.0 * V[4:].sum()

    rn2 = (E.astype(np.float64) ** 2).sum(axis=1)
    T += np.maximum(rn2 - DIAG_CLAMP, 0.0).sum()  # undo diagonal clamp
    frob2 = rn2.sum()
    loss = (T - frob2) / N + LN2
    return np.array(loss, dtype=np.float32)


# revision 2
# speedup vs baseline: 1.2855x; 1.2855x over previous
"""Negative-sampling loss kernel for Trainium2 (8 NeuronCores, SPMD).

Math: for E [16384,128], S = E@E.T,
  loss = (T - frob2)/N + ln(2)
where T = sum_ij softplus(S_ij) (full matrix incl. diagonal),
frob2 = sum(E^2), N = 16384.

S is symmetric: T = P1 + 2*P2 over 512x512 super-blocks (32 diagonal
blocks once, 496 unique off-diagonal blocks twice). Circulant SPMD:
core c owns super-rows {c, c+8, c+16, c+24}; 66 blocks/core, identical
program per core.

Device pipeline (per pair of 512-col groups = [128, 4096] scores):
  bf16 matmuls -> PSUM f32; diagonal groups pre-clamped at 88 (DVE);
  ACT Exp PSUM -> u bf16; DVE min(u, K=e^44); one pairing level
  z = (1+ua)(1+ub)-1 = ua*ub+ua+ub (stt + tt, bf16 2x); ACT
  Ln(z*2^-64 + 2^-64) = softplus(sa)+softplus(sb) - 64*ln2 with
  per-pair accum. Host restores offsets, swaps the (always-clamped)
  diagonal for exact fp64 softplus(|e_i|^2), applies symmetry weights.
Residual bias: off-diagonal scores above 44 are clamped (softplus
excess ~ Sum relu(s-44) ~ 3e4 of T ~ 1.2e9 -> ~3e-5 relative).
"""

import sys

for _p in ("/opt/trn_rl_repo",):
    if _p not in sys.path:
        sys.path.insert(0, _p)

import numpy as np

N = 16384
D = 128
SB = 512            # super-block edge
NSB = N // SB       # 32 super-rows
NCORES = 8
DS = (17, 17, 16, 16)   # blocks per slot (slots 0,1 hold super-rows 0..15)
GROUPS = [(s, 0) for s in range(4)] + [
    (s, d) for s in range(4) for d in range(1, DS[s])
]                   # 66 groups; diagonal blocks first
NG = len(GROUPS)
NPAIR = NG // 2     # 33; pairs 0,1 are the diagonal groups (weight 1)
GROUP_F = 2048      # free elements per group per partition
PAIR_F = 2 * GROUP_F
LN_SCALE = float(2.0 ** -64)
DIAG_CLAMP = 88.0
UCLAMP_S = 44.0
LN2 = float(np.log(2.0))

TRACE = False
LAST_EXEC_NS = None
LAST_PROFILE = None

_COMPILED = None


def _bf16_dtype():
    import ml_dtypes
    return ml_dtypes.bfloat16


def _build():
    import concourse.bacc as bacc
    import concourse.mybir as mybir
    import concourse.hw_specs as hw_specs
    from concourse import tile
    from contextlib import ExitStack

    AF = mybir.ActivationFunctionType
    ALU = mybir.AluOpType

    # Force Exp and Ln to resolve to the single table set containing both,
    # so the table-load pass never reloads tables between instructions.
    _orig = hw_specs.get_activation_tables

    def _patched(arch):
        t = {k: set(v) for k, v in _orig(arch).items()}
        for name in t:
            if name != "natural_log_exp_and_others":
                t[name] -= {AF.Exp, AF.Ln}
        return t

    bacc.get_activation_tables = _patched

    nc = bacc.Bacc("TRN2", target_bir_lowering=False, debug=False,
                   num_devices=NCORES)
    f32 = mybir.dt.float32
    bf16 = mybir.dt.bfloat16
    cols_ap = nc.dram_tensor("cols", [D, NG * SB], bf16,
                             kind="ExternalInput").ap()
    shard_ap = nc.dram_tensor("shard", [D, 4 * SB], bf16,
                              kind="ExternalInput").ap()
    out_ap = nc.dram_tensor("out", [D, NPAIR], f32, kind="ExternalOutput").ap()

    KVAL = float(np.exp(UCLAMP_S))

    with tile.TileContext(nc) as tc:
        with ExitStack() as ctx:
            const_pool = ctx.enter_context(tc.tile_pool(name="const", bufs=1))
            shard_pool = ctx.enter_context(tc.tile_pool(name="shardp", bufs=1))
            rhs_pool = ctx.enter_context(tc.tile_pool(name="rhs", bufs=3))
            u_pool = ctx.enter_context(tc.tile_pool(name="u", bufs=2))
            uc_pool = ctx.enter_context(tc.tile_pool(name="uc", bufs=2))
            t_pool = ctx.enter_context(tc.tile_pool(name="t", bufs=2))
            z_pool = ctx.enter_context(tc.tile_pool(name="z", bufs=2))
            o_pool = ctx.enter_context(tc.tile_pool(name="o", bufs=2))
            acc_pool = ctx.enter_context(tc.tile_pool(name="acc", bufs=1))
            psum_pool = ctx.enter_context(
                tc.tile_pool(name="psum", bufs=2, space="PSUM"))

            bias_t = const_pool.tile([D, 1], f32)
            nc.gpsimd.memset(bias_t[:], LN_SCALE)
            shard_t = shard_pool.tile([D, 4 * SB], bf16)
            nc.sync.dma_start(shard_t[:], shard_ap)
            acc_t = acc_pool.tile([D, NPAIR], f32)

            for p in range(NPAIR):
                rhs_t = rhs_pool.tile([D, 2 * SB], bf16)
                nc.sync.dma_start(
                    rhs_t[:], cols_ap[:, p * 2 * SB:(p + 1) * 2 * SB])
                u_t = u_pool.tile([D, PAIR_F], bf16)
                for g in range(2):
                    s, d = GROUPS[2 * p + g]
                    ps = psum_pool.tile([D, GROUP_F], f32)
                    for m in range(4):
                        lo = s * SB + m * D
                        nc.tensor.matmul(
                            ps[:, m * SB:(m + 1) * SB],
                            shard_t[:, lo:lo + D],
                            rhs_t[:, g * SB:(g + 1) * SB],
                            start=True, stop=True,
                        )
                    if d == 0:
                        nc.vector.tensor_scalar_min(ps[:], ps[:], DIAG_CLAMP)
                    nc.scalar.activation(
                        u_t[:, g * GROUP_F:(g + 1) * GROUP_F], ps[:], AF.Exp)
                uc_t = uc_pool.tile([D, PAIR_F], bf16)
                nc.vector.tensor_scalar_min(uc_t[:], u_t[:], KVAL)
                t_t = t_pool.tile([D, GROUP_F], bf16)
                nc.vector.scalar_tensor_tensor(
                    t_t[:], uc_t[:, :GROUP_F], 1.0, uc_t[:, GROUP_F:],
                    ALU.add, ALU.mult)
                z_t = z_pool.tile([D, GROUP_F], bf16)
                nc.vector.tensor_tensor(
                    z_t[:], t_t[:], uc_t[:, :GROUP_F], ALU.add)
                o_t = o_pool.tile([D, GROUP_F], f32)
                nc.scalar.activation(
                    o_t[:], z_t[:], AF.Ln,
                    bias=bias_t[:], scale=LN_SCALE,
                    accum_out=acc_t[:, p:p + 1],
                )
            nc.sync.dma_start(out_ap, acc_t[:])

    nc.compile()
    return nc


def kernel(doc_embeddings: np.ndarray) -> np.ndarray:
    global _COMPILED, LAST_EXEC_NS, LAST_PROFILE
    E = np.ascontiguousarray(np.asarray(doc_embeddings, dtype=np.float32))
    assert E.shape == (N, D)
    bf = _bf16_dtype()
    ETb = np.ascontiguousarray(E.T.astype(bf))  # [128, 16384] bf16

    if _COMPILED is None:
        _COMPILED = _build()
    nc = _COMPILED

    in_maps = []
    for c in range(NCORES):
        slots = [c, c + 8, c + 16, c + 24]
        cols_c = np.concatenate(
            [ETb[:, ((slots[s] + d) % NSB) * SB:((slots[s] + d) % NSB) * SB + SB]
             for (s, d) in GROUPS], axis=1)
        shard_c = np.concatenate(
            [ETb[:, r * SB:r * SB + SB] for r in slots], axis=1)
        in_maps.append({"cols": np.ascontiguousarray(cols_c),
                        "shard": np.ascontiguousarray(shard_c)})

    from concourse import bass_utils
    res = bass_utils.run_bass_kernel_spmd(
        nc, in_maps, core_ids=list(range(NCORES)), trace=TRACE)
    LAST_EXEC_NS = res.exec_time_ns
    LAST_PROFILE = res.profile_json

    # Each accum column summed 128*2048 Ln outputs, each offset by -64*ln2,
    # each covering TWO scores.
    off = float(D) * GROUP_F * 64.0 * LN2
    T = 0.0
    for c in range(NCORES):
        colsums = res.results[c]["out"].astype(np.float64).sum(axis=0)
        V = colsums + off
        T += V[:2].sum() + 2.0 * V[2:].sum()

    # Swap the (always-clamped) diagonal for the exact fp64 value.
    E64 = E.astype(np.float64)
    rn2 = (E64 ** 2).sum(axis=1)
    KDEV = float(np.float32(np.exp(UCLAMP_S).astype(bf)))
    ln1pK = float(np.log1p(KDEV))
    T += np.logaddexp(0.0, rn2).sum() - N * ln1pK

    frob2 = rn2.sum()
    loss = (T - frob2) / N + LN2
    return np.array(loss, dtype=np.float32)


# revision 9
# speedup vs baseline: 1.6365x; 1.2730x over previous
"""Negative-sampling loss kernel for Trainium2 (8 NeuronCores, SPMD).

Math: for E [16384,128], S = E@E.T,
  loss = (T - frob2)/N + ln(2)
where T = sum_ij softplus(S_ij) (full matrix incl. diagonal),
frob2 = sum(E^2), N = 16384.

S is symmetric: T = P1 + 2*P2 over 512x512 super-blocks (32 diagonal
blocks once, 496 unique off-diagonal blocks twice). Circulant SPMD:
core c owns super-rows {c, c+8, c+16, c+24}; 66 blocks/core, identical
program per core.

Sigma-quad pipeline: matmul with NEGATED shard -> PSUM holds -s;
ACT Sigmoid PSUM->bf16 gives q = sigmoid(-s) in (0,1] (no overflow,
no clamp; table accurate to ~6e-7 down to sigmoid(-88)).
softplus(s) = -ln q. Quads of 4 groups: W = (q0*q1)*(q2*q3) (3 DVE
tt, bf16; no extreme DVE scalars -- those crash the exec unit).
Final pass: ACT Ln(2^63*W + 2^-64) with per-column accumulate (one
table reload); sum softplus = 63*ln2 - ln_out. 2^63*W <= 2^63 < 2^64
stays in the Ln table domain; clamp at W < 2^-127 (quad softplus
sum > 88) ~never happens off-diagonal (P ~ 4e-8).
Diagonal entries (s=|e_i|^2 ~ 128): the 4 diagonal groups form
quad 0 and share diag positions (c = m*128+p independent of slot),
so at each of the 512 diag positions/core all four sigmoids
underflow to 0 together -> W = 0 exactly -> ln_out = ln(2^-64)
= -64*ln2, i.e. the position contributes exactly 127*ln2; the host
swaps that for the fp64 softplus(|e_i|^2) of the 4 entries.
"""

import sys

for _p in ("/opt/trn_rl_repo",):
    if _p not in sys.path:
        sys.path.insert(0, _p)

import numpy as np

N = 16384
D = 128
SB = 512            # super-block edge
NSB = N // SB       # 32 super-rows
NCORES = 8
DS = (17, 17, 16, 16)   # blocks per slot (slots 0,1 hold super-rows 0..15)
GROUPS = [(s, 0) for s in range(4)] + [
    (s, d) for s in range(4) for d in range(1, DS[s])
]                   # 66 groups; the 4 diagonal blocks first (= quad 0)
NG = len(GROUPS)
NQUAD = 16          # quads 0..15 (quad 0 = diagonal); groups 64,65 = pair
GROUP_F = 2048      # free elements per group per partition
QUAD_F = 4 * GROUP_F
W_SLOTS = 17        # 16 quads + 1 pair
LN_BIAS = float(2.0 ** -64)
QSCALE = float(2.0 ** 63)    # quad Ln scale (ACT f32 param); ln = 63*ln2
PSCALE = float(2.0 ** 31)    # pair Ln scale; ln = 31*ln2
LN2 = float(np.log(2.0))

TRACE = False
LAST_EXEC_NS = None
LAST_PROFILE = None

_COMPILED = None


def _bf16_dtype():
    import ml_dtypes
    return ml_dtypes.bfloat16


def _build():
    import concourse.bacc as bacc
    import concourse.mybir as mybir
    import concourse.hw_specs as hw_specs
    from concourse import tile
    from contextlib import ExitStack

    AF = mybir.ActivationFunctionType
    ALU = mybir.AluOpType

    # Pin Sigmoid and Ln each to a single table set so exactly two
    # ACT_TABLE_LOADs are emitted (one per pass).
    _orig = hw_specs.get_activation_tables

    def _patched(arch):
        t = {k: set(v) for k, v in _orig(arch).items()}
        for name in t:
            if name != "natural_log_exp_and_others":
                t[name] -= {AF.Exp, AF.Ln}
            if name != "sigmoid_and_others":
                t[name] -= {AF.Sigmoid}
        return t

    bacc.get_activation_tables = _patched

    nc = bacc.Bacc("TRN2", target_bir_lowering=False, debug=False,
                   num_devices=NCORES)
    f32 = mybir.dt.float32
    bf16 = mybir.dt.bfloat16
    cols_ap = nc.dram_tensor("cols", [D, NG * SB], bf16,
                             kind="ExternalInput").ap()
    shard_ap = nc.dram_tensor("shard", [D, 4 * SB], bf16,
                              kind="ExternalInput").ap()
    out_ap = nc.dram_tensor("out", [D, 6], f32, kind="ExternalOutput").ap()

    with tile.TileContext(nc) as tc:
        with ExitStack() as ctx:
            const_pool = ctx.enter_context(tc.tile_pool(name="const", bufs=1))
            shard_pool = ctx.enter_context(tc.tile_pool(name="shardp", bufs=1))
            rhs_pool = ctx.enter_context(tc.tile_pool(name="rhs", bufs=3))
            sig_pool = ctx.enter_context(tc.tile_pool(name="sig", bufs=2))
            p_pool = ctx.enter_context(tc.tile_pool(name="pp", bufs=4))
            w_pool = ctx.enter_context(tc.tile_pool(name="w", bufs=1))
            o_pool = ctx.enter_context(tc.tile_pool(name="o", bufs=2))
            acc_pool = ctx.enter_context(tc.tile_pool(name="acc", bufs=1))
            psum_pool = ctx.enter_context(
                tc.tile_pool(name="psum", bufs=2, space="PSUM"))

            bias_t = const_pool.tile([D, 1], f32)
            nc.gpsimd.memset(bias_t[:], LN_BIAS)
            shard_t = shard_pool.tile([D, 4 * SB], bf16)
            nc.sync.dma_start(shard_t[:], shard_ap)
            acc_t = acc_pool.tile([D, 6], f32)
            w_t = w_pool.tile([D, W_SLOTS * GROUP_F], bf16)

            def mm_group(ps, g, rhs_t, j):
                s, _ = GROUPS[g]
                for m in range(4):
                    lo = s * SB + m * D
                    nc.tensor.matmul(
                        ps[:, m * SB:(m + 1) * SB],
                        shard_t[:, lo:lo + D],
                        rhs_t[:, j * SB:(j + 1) * SB],
                        start=True, stop=True,
                    )

            for q in range(NQUAD):
                rhs_t = rhs_pool.tile([D, 4 * SB], bf16)
                nc.sync.dma_start(
                    rhs_t[:], cols_ap[:, q * 4 * SB:(q + 1) * 4 * SB])
                sig_t = sig_pool.tile([D, QUAD_F], bf16)
                for j in range(4):
                    ps = psum_pool.tile([D, GROUP_F], f32)
                    mm_group(ps, 4 * q + j, rhs_t, j)
                    nc.scalar.activation(
                        sig_t[:, j * GROUP_F:(j + 1) * GROUP_F], ps[:],
                        AF.Sigmoid)
                p1 = p_pool.tile([D, GROUP_F], bf16)
                nc.vector.tensor_tensor(
                    p1[:], sig_t[:, :GROUP_F], sig_t[:, GROUP_F:2 * GROUP_F],
                    ALU.mult)
                p2 = p_pool.tile([D, GROUP_F], bf16)
                nc.vector.tensor_tensor(
                    p2[:], sig_t[:, 2 * GROUP_F:3 * GROUP_F],
                    sig_t[:, 3 * GROUP_F:], ALU.mult)
                nc.vector.tensor_tensor(
                    w_t[:, q * GROUP_F:(q + 1) * GROUP_F],
                    p1[:], p2[:], ALU.mult)

            # trailing pair: groups 64, 65
            rhs_t = rhs_pool.tile([D, 2 * SB], bf16)
            nc.sync.dma_start(rhs_t[:], cols_ap[:, 64 * SB:66 * SB])
            sig_t = sig_pool.tile([D, 2 * GROUP_F], bf16)
            for j in range(2):
                ps = psum_pool.tile([D, GROUP_F], f32)
                mm_group(ps, 64 + j, rhs_t, j)
                nc.scalar.activation(
                    sig_t[:, j * GROUP_F:(j + 1) * GROUP_F], ps[:],
                    AF.Sigmoid)
            nc.vector.tensor_tensor(
                w_t[:, NQUAD * GROUP_F:(NQUAD + 1) * GROUP_F],
                sig_t[:, :GROUP_F], sig_t[:, GROUP_F:], ALU.mult)

            # Ln pass (one table reload): 6 instructions, 6 accum columns
            ln_slices = [
                (0, GROUP_F, QSCALE),            # quad 0 (diag, weight 1)
                (GROUP_F, 4 * GROUP_F, QSCALE),  # quads 1-15 (weight 2)
                (5 * GROUP_F, 4 * GROUP_F, QSCALE),
                (9 * GROUP_F, 4 * GROUP_F, QSCALE),
                (13 * GROUP_F, 3 * GROUP_F, QSCALE),
                (16 * GROUP_F, GROUP_F, PSCALE),  # pair (weight 2)
            ]
            for ci, (lo, ln_n, sc) in enumerate(ln_slices):
                o_t = o_pool.tile([D, ln_n], bf16)
                nc.scalar.activation(
                    o_t[:], w_t[:, lo:lo + ln_n], AF.Ln,
                    bias=bias_t[:], scale=sc,
                    accum_out=acc_t[:, ci:ci + 1],
                )
            nc.sync.dma_start(out_ap, acc_t[:])

    nc.compile()
    return nc


def kernel(doc_embeddings: np.ndarray) -> np.ndarray:
    global _COMPILED, LAST_EXEC_NS, LAST_PROFILE
    E = np.ascontiguousarray(np.asarray(doc_embeddings, dtype=np.float32))
    assert E.shape == (N, D)
    bf = _bf16_dtype()
    ETb = np.ascontiguousarray(E.T.astype(bf))  # [128, 16384] bf16

    if _COMPILED is None:
        _COMPILED = _build()
    nc = _COMPILED

    in_maps = []
    for c in range(NCORES):
        slots = [c, c + 8, c + 16, c + 24]
        cols_c = np.concatenate(
            [ETb[:, ((slots[s] + d) % NSB) * SB:((slots[s] + d) % NSB) * SB + SB]
             for (s, d) in GROUPS], axis=1)
        shard_c = np.concatenate(
            [-ETb[:, r * SB:r * SB + SB] for r in slots], axis=1)
        in_maps.append({"cols": np.ascontiguousarray(cols_c),
                        "shard": np.ascontiguousarray(shard_c)})

    from concourse import bass_utils
    res = bass_utils.run_bass_kernel_spmd(
        nc, in_maps, core_ids=list(range(NCORES)), trace=TRACE)
    LAST_EXEC_NS = res.exec_time_ns
    LAST_PROFILE = res.profile_json

    # Each Ln value = ln(W + 2^-64) = k*ln2 - sum_softplus(quad elems).
    # col0: quad0 [2048/pt] k=63 w=1; cols1-4: quads1-15 k=63 w=2;
    # col5: pair [2048/pt] k=31 w=2.
    T = 0.0
    for c in range(NCORES):
        V = res.results[c]["out"].astype(np.float64).sum(axis=0)
        S0 = D * GROUP_F * 63.0 * LN2 - V[0]
        Smid = D * 15 * GROUP_F * 63.0 * LN2 - (V[1] + V[2] + V[3] + V[4])
        Spair = D * GROUP_F * 31.0 * LN2 - V[5]
        T += S0 + 2.0 * Smid + 2.0 * Spair

    # Each of the NCORES*SB diag positions has all 4 sigmoids = 0 ->
    # device softplus-sum exactly 127*ln2; swap for fp64 softplus of the
    # 4 true-diagonal entries it covers.
    E64 = E.astype(np.float64)
    rn2 = (E64 ** 2).sum(axis=1)
    T += np.logaddexp(0.0, rn2).sum() - NCORES * SB * 127.0 * LN2

    frob2 = rn2.sum()
    loss = (T - frob2) / N + LN2
    return np.array(loss, dtype=np.float32)
